# revision 11
# baseline (speedup 1.0000x reference)
"""MultiHeadEMA on 8 Trainium2 NeuronCores.

Strategy
--------
Channel-sharded: embed_dim=1024 -> 8 slices of 128 channels (= SBUF
partitions), one per core. The reference's FFT conv is exactly an order-2 IIR
    y_n[l] = q_n y_n[l-1] + x[l],   out = silu(c0 y0 + c1 y1 + omega x)
computed with `tensor_tensor_scan` on the vector engine, decimated by 4:
    Y_n[j] = y_n[4j],  Y_n[j] = q_n^4 Y_n[j-1] + u_n[j]
    u_n[j] = x[4j] + q_n x[4j-1] + q_n^2 x[4j-2] + q_n^3 x[4j-3]
u_n is built by accumulating diagonal matmuls (tensor engine, bf16) into PSUM
from contiguous phase blocks of x (deinterleaved / pre-shifted on the host).
Phase outputs r>=1 expand into diagonal matmuls over (Y0, Y1, phase blocks)
accumulated in PSUM, evacuated by Silu on the scalar engine. Phase 0
(out_0 = Y'0 + Y'1 + w x0) is assembled on the vector engine in bf16 and
silu'd straight from SBUF, keeping it off the tensor engine.

v3 scheduling (v1 67.2us, v2 61.9us; PE streams 512-col bf16 MMs at 216 ns
warm so the whole game is keeping the MM stream dense and starting it early):
  * host precomputes the 20 per-channel coefficient columns; coef+eye ship
    as ONE small DMA (bitcast view) so diag-weight building starts ~+9us.
  * u-diags built on DVE, out-diags on ACT, in parallel.
  * dummy warm-up matmuls on zeroed tiles run during the DMA head so the PE
    HAM clock-gate is at 8/8 when real MMs start.
  * batch-0 x DMA split in halves, first half issued on the Scalar HWDGE
    queue in parallel with the const DMA on the Sync queue.
  * software pipelining: u(b+1) matmuls interleave between out(b) phase
    groups; scans are chained 512-col halves so Y is available early.
  * PSUM is split into per-bank tiles (Tile's dependency tracker is
    tile-granular): 4x u half-banks, a 2-bank (r1,r2) tile and a 1-bank r3
    tile, so silu evacuation never stalls the next matmul group.
"""

import numpy as np
import ml_dtypes

import concourse.bass as bass
import concourse.bacc as bacc
import concourse.tile as tile
from concourse import mybir
from concourse.bass_utils import run_bass_kernel_spmd

SEQ_LEN, BSZ, EMBED_DIM, NDIM = 4096, 4, 1024, 2
N_CORES = 8
D_PER = EMBED_DIM // N_CORES  # 128 channels/core = full SBUF partitions
SCALE = (1.0 / NDIM) ** 0.5
DEC = 4                   # decimation factor
J = SEQ_LEN // DEC        # decimated length 1024
CH = 512                  # matmul chunk (one fp32 PSUM bank)
NG = J // CH              # j-groups per slab (2)
F32 = mybir.dt.float32
BF16 = mybir.dt.bfloat16
AF = mybir.ActivationFunctionType
ALU = mybir.AluOpType
NBLK = 4                  # x phase blocks r=0..3 -> x[4j+r]
NWARM = 7                 # HAM warm-up matmuls

# coef columns (host precomputed, fp32), see make_in_maps
NCOEF = 20
CST_W = 2 * NCOEF + D_PER  # merged const tensor width in bf16 units


def build_bass():
    nc = bacc.Bacc(name="multihead_ema_v3")
    x = nc.dram_tensor("x", [D_PER, BSZ, NBLK, J], BF16, kind="ExternalInput")
    cst = nc.dram_tensor("cst", [D_PER, CST_W], BF16, kind="ExternalInput")
    out = nc.dram_tensor("out", [D_PER, BSZ, DEC, J], BF16, kind="ExternalOutput")

    with tile.TileContext(nc) as tc:
        with (
            tc.tile_pool(name="const", bufs=1) as const,
            tc.tile_pool(name="xup", bufs=4) as xup,
            tc.tile_pool(name="yp", bufs=2) as yp,
            tc.tile_pool(name="r0p", bufs=2) as r0p,
            tc.tile_pool(name="op", bufs=2) as op,
            tc.tile_pool(name="psu", bufs=4, space="PSUM") as psu,
            tc.tile_pool(name="ps2", bufs=2, space="PSUM") as ps2,
            tc.tile_pool(name="ps3", bufs=2, space="PSUM") as ps3,
        ):
            # --- warm-up operands (memset on gpsimd; no DMA dependency)
            wz = const.tile([D_PER, D_PER], BF16, tag="wz")
            nc.gpsimd.memset(wz[:, :], 0)
            mz = const.tile([D_PER, CH], BF16, tag="mz")
            nc.gpsimd.memset(mz[:, :], 0)

            # --- input DMAs: cst + first x half on sync queue; the second
            # half and odd batches stream in parallel on the scalar queue
            cstsb = const.tile([D_PER, CST_W], BF16)
            nc.sync.dma_start(out=cstsb[:, :], in_=cst[:, :])
            csb = cstsb[:, 0 : 2 * NCOEF].bitcast(F32)   # [128, NCOEF] fp32
            eyesb = cstsb[:, 2 * NCOEF : CST_W]          # [128, 128] bf16
            xus = []
            for b in range(BSZ):
                xu = xup.tile([D_PER, NBLK, J], BF16, tag="xu")
                xus.append(xu)
            nc.sync.dma_start(out=xus[0][:, :, 0:CH], in_=x[:, 0, :, 0:CH])
            nc.scalar.dma_start(out=xus[0][:, :, CH:J], in_=x[:, 0, :, CH:J])
            nc.scalar.dma_start(out=xus[1][:, :, :], in_=x[:, 1, :, :])
            nc.sync.dma_start(out=xus[2][:, :, :], in_=x[:, 2, :, :])
            nc.scalar.dma_start(out=xus[3][:, :, :], in_=x[:, 3, :, :])

            # --- HAM warm-up: one accumulation group of dummy matmuls
            wps = psu.tile([D_PER, CH], F32, tag="u")
            for i in range(NWARM):
                nc.tensor.matmul(wps[:, :], wz[:, :], mz[:, :],
                                 start=(i == 0), stop=(i == NWARM - 1))

            # --- diag weight matrices: u-diags on DVE, out-diags on ACT
            def diag_dve(col, tg):
                t = const.tile([D_PER, D_PER], BF16, tag=tg)
                nc.vector.tensor_scalar_mul(out=t[:, :], in0=eyesb[:, :],
                                            scalar1=csb[:, col : col + 1])
                return t

            def diag_act(col, tg):
                t = const.tile([D_PER, D_PER], BF16, tag=tg)
                nc.scalar.activation(out=t[:, :], in_=eyesb[:, :], func=AF.Copy,
                                     scale=csb[:, col : col + 1])
                return t

            # u-synthesis weights, ordered by first use (n=0 taps, n=1 taps)
            w_u = [[None] * 4, [None] * 4]
            for n in range(NDIM):
                for k in range(4):
                    w_u[n][k] = diag_dve(2 + 2 * k + n, f"wu{n}{k}")
            # out-stage weights, ordered by first use in outg
            # (q^1 diags last: only the final batch computes r=1 on the PE)
            w_q = [[None] * NDIM for _ in range(3)]  # [r-1][n] : diag(q_n^r)
            w_cw = diag_act(17, "wcw")    # csum
            w_q[1][0] = diag_act(12, "wq20")
            w_q[1][1] = diag_act(13, "wq21")
            w_cqs = diag_act(18, "wcqs")
            w_q[2][0] = diag_act(14, "wq30")
            w_q[2][1] = diag_act(15, "wq31")
            w_cq2s = diag_act(19, "wcq2s")
            w_q[0][0] = diag_act(10, "wq10")
            w_q[0][1] = diag_act(11, "wq11")

            q4b = [csb[:, n : n + 1].to_broadcast([D_PER, CH]) for n in range(NDIM)]

            pus = {}   # (b, n, g) -> psum half tile
            Ys = {}    # (b, n) -> sbuf bf16 tile
            obs = {}   # b -> output tile

            def issue_u(b, n):
                """8 diagonal matmuls accumulating u_n for batch b into two
                single-bank PSUM tiles."""
                xu = xus[b]
                for g in range(NG):
                    pu = psu.tile([D_PER, CH], F32, tag="u")
                    pus[(b, n, g)] = pu
                    s = bass.ts(g, CH)
                    nc.tensor.matmul(pu[:, :], w_u[n][0][:, :], xu[:, 0, s],
                                     start=True, stop=False)
                    for k in range(1, 4):  # + c_n q^k * x[4j-k]
                        if g == 0:
                            nc.tensor.matmul(
                                pu[:, 1:CH], w_u[n][k][:, :],
                                xu[:, 4 - k, 0 : CH - 1],
                                start=False, stop=(k == 3))
                        else:
                            nc.tensor.matmul(
                                pu[:, :], w_u[n][k][:, :],
                                xu[:, 4 - k, g * CH - 1 : (g + 1) * CH - 1],
                                start=False, stop=(k == 3))

            def issue_scan(b, n):
                """Chained half scans: Y'_n available per 512-col half."""
                yn = yp.tile([D_PER, J], BF16, tag=f"y{n}")
                Ys[(b, n)] = yn
                nc.vector.tensor_tensor_scan(
                    out=yn[:, 0:CH], data0=q4b[n],
                    data1=pus.pop((b, n, 0))[:, :],
                    initial=0.0, op0=ALU.mult, op1=ALU.add)
                nc.vector.tensor_tensor_scan(
                    out=yn[:, CH:J], data0=q4b[n],
                    data1=pus.pop((b, n, 1))[:, :],
                    initial=yn[:, CH - 1 : CH], op0=ALU.mult, op1=ALU.add)

            def issue_r0(b):
                """Phase 0 fully off the tensor engine:
                out_0 = silu(Y'0 + Y'1 + w*x0), assembled in bf16 on DVE."""
                ob = op.tile([D_PER, DEC, J], BF16)
                obs[b] = ob
                ysum = r0p.tile([D_PER, J], BF16, tag="ys")
                nc.vector.tensor_add(out=ysum[:, :], in0=Ys[(b, 0)][:, :],
                                     in1=Ys[(b, 1)][:, :])
                xw = r0p.tile([D_PER, J], BF16, tag="xw")
                nc.vector.tensor_scalar_mul(out=xw[:, :], in0=xus[b][:, 0, :],
                                            scalar1=csb[:, 16:17])
                pre0 = r0p.tile([D_PER, J], BF16, tag="p0")
                nc.vector.tensor_add(out=pre0[:, :], in0=ysum[:, :], in1=xw[:, :])
                nc.scalar.activation(out=ob[:, 0, :], in_=pre0[:, :], func=AF.Silu)
                nc.sync.dma_start(out=out[:, b, 0, :], in_=ob[:, 0, :])

            r1t = {}

            def issue_r1_scales(b):
                """Phase 1 off the tensor engine (batches 0..2):
                out_1 = silu(q0 Y'0 + q1 Y'1 + csum x1); per-channel scaled
                copies on ACT, adds on the otherwise-idle GpSimd."""
                ya = r0p.tile([D_PER, J], BF16, tag="ya")
                nc.scalar.activation(out=ya[:, :], in_=Ys[(b, 0)][:, :],
                                     func=AF.Copy, scale=csb[:, 10:11])
                yb = r0p.tile([D_PER, J], BF16, tag="yb")
                nc.scalar.activation(out=yb[:, :], in_=Ys[(b, 1)][:, :],
                                     func=AF.Copy, scale=csb[:, 11:12])
                xc = r0p.tile([D_PER, J], BF16, tag="xc")
                nc.gpsimd.tensor_scalar_mul(out=xc[:, :], in0=xus[b][:, 1, :],
                                            scalar1=csb[:, 17:18])
                s1 = r0p.tile([D_PER, J], BF16, tag="s1")
                nc.gpsimd.tensor_add(out=s1[:, :], in0=ya[:, :], in1=yb[:, :])
                pre1 = r0p.tile([D_PER, J], BF16, tag="p1")
                nc.gpsimd.tensor_add(out=pre1[:, :], in0=s1[:, :], in1=xc[:, :])
                r1t[b] = pre1

            def issue_r1_fin(b):
                nc.scalar.activation(out=obs[b][:, 1, :], in_=r1t.pop(b)[:, :],
                                     func=AF.Silu)
                nc.sync.dma_start(out=out[:, b, 1, :], in_=obs[b][:, 1, :])

            def issue_outg(b, g, with_r1):
                """Phases 2,3 (and 1 for the final batch, on spare u-banks)
                for j-group g: diag matmuls into per-bank PSUM tiles, silu
                evacuation, output DMA."""
                xu = xus[b]
                s = bass.ts(g, CH)
                rs = (1, 2, 3) if with_r1 else (2, 3)
                tgts = {}
                for r in rs:
                    if r == 1:
                        pt1 = psu.tile([D_PER, CH], F32, tag="u")
                        tgts[1] = pt1
                    elif r == 2:
                        pt2 = ps2.tile([D_PER, CH], F32, tag="c2")
                        tgts[2] = pt2
                    else:
                        pt3 = ps3.tile([D_PER, CH], F32, tag="c3")
                        tgts[3] = pt3
                for r in rs:
                    tgt = tgts[r][:, :]
                    nc.tensor.matmul(tgt, w_q[r - 1][0][:, :], Ys[(b, 0)][:, s],
                                     start=True, stop=False)
                    nc.tensor.matmul(tgt, w_q[r - 1][1][:, :], Ys[(b, 1)][:, s],
                                     start=False, stop=False)
                    xw = [(w_cw, r)]
                    if r == 2:
                        xw.append((w_cqs, 1))
                    elif r == 3:
                        xw.append((w_cqs, 2))
                        xw.append((w_cq2s, 1))
                    for i, (wt, rr) in enumerate(xw):
                        nc.tensor.matmul(tgt, wt[:, :], xu[:, rr, s],
                                         start=False, stop=(i == len(xw) - 1))
                for r in rs:
                    nc.scalar.activation(out=obs[b][:, r, s], in_=tgts[r][:, :],
                                         func=AF.Silu)
                    if with_r1:
                        nc.sync.dma_start(out=out[:, b, r, s],
                                          in_=obs[b][:, r, s])
                if not with_r1:
                    nc.sync.dma_start(out=out[:, b, 2:4, s],
                                      in_=obs[b][:, 2:4, s])

            # --- software-pipelined main loop
            issue_u(0, 0)
            issue_scan(0, 0)
            issue_u(0, 1)
            issue_scan(0, 1)
            issue_r1_scales(0)
            issue_r0(0)
            issue_r1_fin(0)
            for b in range(BSZ):
                last = b == BSZ - 1
                if not last:
                    issue_u(b + 1, 0)
                    issue_scan(b + 1, 0)
                issue_outg(b, 0, with_r1=last)
                if not last:
                    issue_u(b + 1, 1)
                    issue_scan(b + 1, 1)
                issue_outg(b, 1, with_r1=last)
                if not last and b + 1 < BSZ - 1:
                    issue_r1_scales(b + 1)
                    issue_r0(b + 1)
                    issue_r1_fin(b + 1)
                elif not last:
                    issue_r0(b + 1)

    nc.compile()
    return nc


_CACHE: dict = {}


def _get_nc():
    if "nc" not in _CACHE:
        _CACHE["nc"] = build_bass()
    return _CACHE["nc"]


def _sigmoid64(a):
    return 1.0 / (1.0 + np.exp(-a.astype(np.float64)))


def make_in_maps(inputs):
    x = np.asarray(inputs["x"], np.float32)
    delta = np.asarray(inputs["delta"], np.float32).reshape(EMBED_DIM, NDIM)
    alpha = np.asarray(inputs["alpha"], np.float32).reshape(EMBED_DIM, NDIM)
    beta = np.asarray(inputs["beta"], np.float32).reshape(EMBED_DIM, NDIM)
    gamma = np.asarray(inputs["gamma"], np.float32).reshape(EMBED_DIM, NDIM)
    omega = np.asarray(inputs["omega"], np.float32).reshape(EMBED_DIM, 1)

    # per-channel coefficient columns (host precompute = weight repacking):
    #  0:2 q^4 | 2:4 c | 4:6 cq | 6:8 cq^2 | 8:10 cq^3 | 10:12 q | 12:14 q^2
    #  14:16 q^3 | 16 w | 17 csum | 18 cqs | 19 cq2s
    p = _sigmoid64(delta)
    q = 1.0 - p * _sigmoid64(alpha)                      # [D, N] f64
    cc = p * beta.astype(np.float64) * gamma.astype(np.float64) * SCALE
    cols = np.empty((EMBED_DIM, NCOEF), np.float64)
    cols[:, 0:2] = q ** 4
    cols[:, 2:4] = cc
    cols[:, 4:6] = cc * q
    cols[:, 6:8] = cc * q ** 2
    cols[:, 8:10] = cc * q ** 3
    cols[:, 10:12] = q
    cols[:, 12:14] = q ** 2
    cols[:, 14:16] = q ** 3
    cols[:, 16:17] = omega
    cols[:, 17:18] = cc.sum(1, keepdims=True) + omega
    cols[:, 18:19] = (cc * q).sum(1, keepdims=True)
    cols[:, 19:20] = (cc * q ** 2).sum(1, keepdims=True)
    coef_full = np.ascontiguousarray(cols.astype(np.float32))

    eye = np.eye(D_PER, dtype=ml_dtypes.bfloat16)
    in_maps = []
    for c in range(N_CORES):
        sl = slice(c * D_PER, (c + 1) * D_PER)
        xc = x[:, :, sl].transpose(2, 1, 0).astype(ml_dtypes.bfloat16)  # [128,B,L]
        ph = xc.reshape(D_PER, BSZ, J, DEC).transpose(0, 1, 3, 2)  # [128,B,4,J]
        cst = np.empty((D_PER, CST_W), dtype=ml_dtypes.bfloat16)
        cst[:, 0 : 2 * NCOEF] = coef_full[sl].view(ml_dtypes.bfloat16)
        cst[:, 2 * NCOEF :] = eye
        in_maps.append({"x": np.ascontiguousarray(ph), "cst": cst})
    return in_maps


def gather_out(results):
    out = np.empty((SEQ_LEN, BSZ, EMBED_DIM), np.float32)
    for c in range(N_CORES):
        # [128, B, 4, J] phase-major -> [l = 4j+r, b, d]
        arr = results[c]["out"].astype(np.float32)
        out[:, :, c * D_PER : (c + 1) * D_PER] = arr.transpose(3, 2, 1, 0).reshape(
            SEQ_LEN, BSZ, D_PER
        )
    return out


def _run(inputs, **kwargs):
    nc = _get_nc()
    in_maps = make_in_maps(inputs)
    res = run_bass_kernel_spmd(nc, in_maps, core_ids=list(range(N_CORES)), **kwargs)
    return gather_out(res.results), res


def kernel(**inputs) -> np.ndarray:
    out, _ = _run(inputs)
    return out


# revision 14
# speedup vs baseline: 1.5254x; 1.5254x over previous
"""MultiHeadEMA on 8 Trainium2 NeuronCores.

Strategy
--------
Channel-sharded: embed_dim=1024 -> 8 slices of 128 channels (= SBUF
partitions), one per core. The reference's FFT conv is exactly an order-2 IIR
    y_n[l] = q_n y_n[l-1] + x[l],   out = silu(c0 y0 + c1 y1 + omega x)
computed with `tensor_tensor_scan` on the vector engine, decimated by 4:
    Y_n[j] = y_n[4j],  Y_n[j] = q_n^4 Y_n[j-1] + u_n[j]
    u_n[j] = x[4j] + q_n x[4j-1] + q_n^2 x[4j-2] + q_n^3 x[4j-3]
u_n is built by accumulating diagonal matmuls (tensor engine, bf16) into PSUM
from contiguous phase blocks of x (deinterleaved / pre-shifted on the host).
Phase outputs r>=1 expand into diagonal matmuls over (Y0, Y1, phase blocks)
accumulated in PSUM, evacuated by Silu on the scalar engine. Phase 0
(out_0 = Y'0 + Y'1 + w x0) is assembled on the vector engine in bf16 and
silu'd straight from SBUF, keeping it off the tensor engine.

v3 scheduling (v1 67.2us, v2 61.9us; PE streams 512-col bf16 MMs at 216 ns
warm so the whole game is keeping the MM stream dense and starting it early):
  * host precomputes the 20 per-channel coefficient columns; coef+eye ship
    as ONE small DMA (bitcast view) so diag-weight building starts ~+9us.
  * u-diags built on DVE, out-diags on ACT, in parallel.
  * dummy warm-up matmuls on zeroed tiles run during the DMA head so the PE
    HAM clock-gate is at 8/8 when real MMs start.
  * batch-0 x DMA split in halves, first half issued on the Scalar HWDGE
    queue in parallel with the const DMA on the Sync queue.
  * software pipelining: u(b+1) matmuls interleave between out(b) phase
    groups; scans are chained 512-col halves so Y is available early.
  * PSUM is split into per-bank tiles (Tile's dependency tracker is
    tile-granular): 4x u half-banks, a 2-bank (r1,r2) tile and a 1-bank r3
    tile, so silu evacuation never stalls the next matmul group.
"""

import numpy as np
import ml_dtypes

import concourse.bass as bass
import concourse.bacc as bacc
import concourse.tile as tile
from concourse import mybir
from concourse.bass_utils import run_bass_kernel_spmd

SEQ_LEN, BSZ, EMBED_DIM, NDIM = 4096, 4, 1024, 2
N_CORES = 8
D_PER = EMBED_DIM // N_CORES  # 128 channels/core = full SBUF partitions
SCALE = (1.0 / NDIM) ** 0.5
DEC = 4                   # decimation factor
J = SEQ_LEN // DEC        # decimated length 1024
CH = 512                  # matmul chunk (one fp32 PSUM bank)
NG = J // CH              # j-groups per slab (2)
F32 = mybir.dt.float32
BF16 = mybir.dt.bfloat16
AF = mybir.ActivationFunctionType
ALU = mybir.AluOpType
NBLK = 4                  # x phase blocks r=0..3 -> x[4j+r]
NWARM = 7                 # HAM warm-up matmuls

# coef columns (host precomputed, fp32), see make_in_maps
NCOEF = 20
CST_W = 2 * NCOEF + D_PER  # merged const tensor width in bf16 units


def build_bass():
    nc = bacc.Bacc(name="multihead_ema_v3")
    x = nc.dram_tensor("x", [D_PER, BSZ, NBLK, J], BF16, kind="ExternalInput")
    cst = nc.dram_tensor("cst", [D_PER, CST_W], BF16, kind="ExternalInput")
    out = nc.dram_tensor("out", [D_PER, BSZ, DEC, J], BF16, kind="ExternalOutput")

    with tile.TileContext(nc) as tc:
        with (
            tc.tile_pool(name="const", bufs=1) as const,
            tc.tile_pool(name="xup", bufs=4) as xup,
            tc.tile_pool(name="yp", bufs=2) as yp,
            tc.tile_pool(name="r0p", bufs=2) as r0p,
            tc.tile_pool(name="op", bufs=2) as op,
            tc.tile_pool(name="psu", bufs=4, space="PSUM") as psu,
            tc.tile_pool(name="ps12", bufs=1, space="PSUM") as ps12,
            tc.tile_pool(name="ps3", bufs=2, space="PSUM") as ps3,
        ):
            # --- warm-up operands (memset on gpsimd; no DMA dependency)
            wz = const.tile([D_PER, D_PER], BF16, tag="wz")
            nc.gpsimd.memset(wz[:, :], 0)
            mz = const.tile([D_PER, CH], BF16, tag="mz")
            nc.gpsimd.memset(mz[:, :], 0)

            # --- input DMAs: cst + first x half on sync queue; the second
            # half and odd batches stream in parallel on the scalar queue
            cstsb = const.tile([D_PER, CST_W], BF16)
            nc.sync.dma_start(out=cstsb[:, :], in_=cst[:, :])
            csb = cstsb[:, 0 : 2 * NCOEF].bitcast(F32)   # [128, NCOEF] fp32
            eyesb = cstsb[:, 2 * NCOEF : CST_W]          # [128, 128] bf16
            xus = []
            for b in range(BSZ):
                xu = xup.tile([D_PER, NBLK, J], BF16, tag="xu")
                xus.append(xu)
            nc.sync.dma_start(out=xus[0][:, :, 0:CH], in_=x[:, 0, :, 0:CH])
            nc.scalar.dma_start(out=xus[0][:, :, CH:J], in_=x[:, 0, :, CH:J])
            nc.scalar.dma_start(out=xus[1][:, :, :], in_=x[:, 1, :, :])
            nc.sync.dma_start(out=xus[2][:, :, :], in_=x[:, 2, :, :])
            nc.scalar.dma_start(out=xus[3][:, :, :], in_=x[:, 3, :, :])

            # --- HAM warm-up: one accumulation group of dummy matmuls
            wps = psu.tile([D_PER, CH], F32, tag="u")
            for i in range(NWARM):
                nc.tensor.matmul(wps[:, :], wz[:, :], mz[:, :],
                                 start=(i == 0), stop=(i == NWARM - 1))

            # --- diag weight matrices: u-diags on DVE, out-diags on ACT
            def diag_dve(col, tg):
                t = const.tile([D_PER, D_PER], BF16, tag=tg)
                nc.vector.tensor_scalar_mul(out=t[:, :], in0=eyesb[:, :],
                                            scalar1=csb[:, col : col + 1])
                return t

            def diag_act(col, tg):
                t = const.tile([D_PER, D_PER], BF16, tag=tg)
                nc.scalar.activation(out=t[:, :], in_=eyesb[:, :], func=AF.Copy,
                                     scale=csb[:, col : col + 1])
                return t

            # u-synthesis weights, ordered by first use (n=0 taps, n=1 taps)
            w_u = [[None] * 4, [None] * 4]
            for n in range(NDIM):
                for k in range(4):
                    w_u[n][k] = diag_dve(2 + 2 * k + n, f"wu{n}{k}")
            # out-stage weights, ordered by first use in outg
            # (q^1 diags last: only the final batch computes r=1 on the PE)
            w_q = [[None] * NDIM for _ in range(3)]  # [r-1][n] : diag(q_n^r)
            w_cw = diag_act(17, "wcw")    # csum
            w_q[1][0] = diag_act(12, "wq20")
            w_q[1][1] = diag_act(13, "wq21")
            w_cqs = diag_act(18, "wcqs")
            w_q[2][0] = diag_act(14, "wq30")
            w_q[2][1] = diag_act(15, "wq31")
            w_cq2s = diag_act(19, "wcq2s")
            w_q[0][0] = diag_act(10, "wq10")
            w_q[0][1] = diag_act(11, "wq11")

            q4b = [csb[:, n : n + 1].to_broadcast([D_PER, CH]) for n in range(NDIM)]

            pus = {}   # (b, n, g) -> psum half tile
            Ys = {}    # (b, n) -> sbuf bf16 tile
            obs = {}   # b -> output tile

            def issue_u(b, n):
                """8 diagonal matmuls accumulating u_n for batch b into two
                single-bank PSUM tiles."""
                xu = xus[b]
                for g in range(NG):
                    pu = psu.tile([D_PER, CH], F32, tag="u")
                    pus[(b, n, g)] = pu
                    s = bass.ts(g, CH)
                    nc.tensor.matmul(pu[:, :], w_u[n][0][:, :], xu[:, 0, s],
                                     start=True, stop=False)
                    for k in range(1, 4):  # + c_n q^k * x[4j-k]
                        if g == 0:
                            nc.tensor.matmul(
                                pu[:, 1:CH], w_u[n][k][:, :],
                                xu[:, 4 - k, 0 : CH - 1],
                                start=False, stop=(k == 3))
                        else:
                            nc.tensor.matmul(
                                pu[:, :], w_u[n][k][:, :],
                                xu[:, 4 - k, g * CH - 1 : (g + 1) * CH - 1],
                                start=False, stop=(k == 3))

            def issue_scan(b, n):
                """Chained half scans: Y'_n available per 512-col half."""
                yn = yp.tile([D_PER, J], BF16, tag=f"y{n}")
                Ys[(b, n)] = yn
                nc.vector.tensor_tensor_scan(
                    out=yn[:, 0:CH], data0=q4b[n],
                    data1=pus.pop((b, n, 0))[:, :],
                    initial=0.0, op0=ALU.mult, op1=ALU.add)
                nc.vector.tensor_tensor_scan(
                    out=yn[:, CH:J], data0=q4b[n],
                    data1=pus.pop((b, n, 1))[:, :],
                    initial=yn[:, CH - 1 : CH], op0=ALU.mult, op1=ALU.add)

            def issue_r0(b):
                """Phase 0 fully off the tensor engine:
                out_0 = silu(Y'0 + Y'1 + w*x0), assembled in bf16 on DVE."""
                ob = op.tile([D_PER, DEC, J], BF16)
                obs[b] = ob
                ysum = r0p.tile([D_PER, J], BF16, tag="ys")
                nc.vector.tensor_add(out=ysum[:, :], in0=Ys[(b, 0)][:, :],
                                     in1=Ys[(b, 1)][:, :])
                xw = r0p.tile([D_PER, J], BF16, tag="xw")
                nc.vector.tensor_scalar_mul(out=xw[:, :], in0=xus[b][:, 0, :],
                                            scalar1=csb[:, 16:17])
                pre0 = r0p.tile([D_PER, J], BF16, tag="p0")
                nc.vector.tensor_add(out=pre0[:, :], in0=ysum[:, :], in1=xw[:, :])
                nc.scalar.activation(out=ob[:, 0, :], in_=pre0[:, :], func=AF.Silu)
                nc.sync.dma_start(out=out[:, b, 0, :], in_=ob[:, 0, :])

            def issue_outg(b, g, split_dma):
                """Phases 1..3 for j-group g: 12 diag matmuls into per-bank
                PSUM tiles, silu evacuation, output DMA."""
                xu = xus[b]
                s = bass.ts(g, CH)
                pt12 = ps12.tile([D_PER, 2, CH], F32, tag="c12")
                pt3 = ps3.tile([D_PER, CH], F32, tag="c3")
                for r in (1, 2, 3):
                    tgt = pt3[:, :] if r == 3 else pt12[:, r - 1, :]
                    nc.tensor.matmul(tgt, w_q[r - 1][0][:, :], Ys[(b, 0)][:, s],
                                     start=True, stop=False)
                    nc.tensor.matmul(tgt, w_q[r - 1][1][:, :], Ys[(b, 1)][:, s],
                                     start=False, stop=False)
                    xw = [(w_cw, r)]
                    if r == 2:
                        xw.append((w_cqs, 1))
                    elif r == 3:
                        xw.append((w_cqs, 2))
                        xw.append((w_cq2s, 1))
                    for i, (wt, rr) in enumerate(xw):
                        nc.tensor.matmul(tgt, wt[:, :], xu[:, rr, s],
                                         start=False, stop=(i == len(xw) - 1))
                nc.scalar.activation(out=obs[b][:, 1:3, s], in_=pt12[:, :, :],
                                     func=AF.Silu)
                nc.scalar.activation(out=obs[b][:, 3, s], in_=pt3[:, :],
                                     func=AF.Silu)
                if split_dma:
                    nc.sync.dma_start(out=out[:, b, 1:3, s],
                                      in_=obs[b][:, 1:3, s])
                    nc.sync.dma_start(out=out[:, b, 3, s], in_=obs[b][:, 3, s])
                else:
                    nc.sync.dma_start(out=out[:, b, 1:4, s],
                                      in_=obs[b][:, 1:4, s])

            # --- software-pipelined main loop
            issue_u(0, 0)
            issue_scan(0, 0)
            issue_u(0, 1)
            issue_scan(0, 1)
            issue_r0(0)
            for b in range(BSZ):
                last = b == BSZ - 1
                if not last:
                    issue_u(b + 1, 0)
                    issue_scan(b + 1, 0)
                issue_outg(b, 0, split_dma=last)
                if not last:
                    issue_u(b + 1, 1)
                    issue_scan(b + 1, 1)
                issue_outg(b, 1, split_dma=last)
                if not last:
                    issue_r0(b + 1)

    nc.compile()
    return nc


_CACHE: dict = {}


def _get_nc():
    if "nc" not in _CACHE:
        _CACHE["nc"] = build_bass()
    return _CACHE["nc"]


def _sigmoid64(a):
    return 1.0 / (1.0 + np.exp(-a.astype(np.float64)))


def make_in_maps(inputs):
    x = np.asarray(inputs["x"], np.float32)
    delta = np.asarray(inputs["delta"], np.float32).reshape(EMBED_DIM, NDIM)
    alpha = np.asarray(inputs["alpha"], np.float32).reshape(EMBED_DIM, NDIM)
    beta = np.asarray(inputs["beta"], np.float32).reshape(EMBED_DIM, NDIM)
    gamma = np.asarray(inputs["gamma"], np.float32).reshape(EMBED_DIM, NDIM)
    omega = np.asarray(inputs["omega"], np.float32).reshape(EMBED_DIM, 1)

    # per-channel coefficient columns (host precompute = weight repacking):
    #  0:2 q^4 | 2:4 c | 4:6 cq | 6:8 cq^2 | 8:10 cq^3 | 10:12 q | 12:14 q^2
    #  14:16 q^3 | 16 w | 17 csum | 18 cqs | 19 cq2s
    p = _sigmoid64(delta)
    q = 1.0 - p * _sigmoid64(alpha)                      # [D, N] f64
    cc = p * beta.astype(np.float64) * gamma.astype(np.float64) * SCALE
    cols = np.empty((EMBED_DIM, NCOEF), np.float64)
    cols[:, 0:2] = q ** 4
    cols[:, 2:4] = cc
    cols[:, 4:6] = cc * q
    cols[:, 6:8] = cc * q ** 2
    cols[:, 8:10] = cc * q ** 3
    cols[:, 10:12] = q
    cols[:, 12:14] = q ** 2
    cols[:, 14:16] = q ** 3
    cols[:, 16:17] = omega
    cols[:, 17:18] = cc.sum(1, keepdims=True) + omega
    cols[:, 18:19] = (cc * q).sum(1, keepdims=True)
    cols[:, 19:20] = (cc * q ** 2).sum(1, keepdims=True)
    coef_full = np.ascontiguousarray(cols.astype(np.float32))

    eye = np.eye(D_PER, dtype=ml_dtypes.bfloat16)
    in_maps = []
    for c in range(N_CORES):
        sl = slice(c * D_PER, (c + 1) * D_PER)
        xc = x[:, :, sl].transpose(2, 1, 0).astype(ml_dtypes.bfloat16)  # [128,B,L]
        ph = xc.reshape(D_PER, BSZ, J, DEC).transpose(0, 1, 3, 2)  # [128,B,4,J]
        cst = np.empty((D_PER, CST_W), dtype=ml_dtypes.bfloat16)
        cst[:, 0 : 2 * NCOEF] = coef_full[sl].view(ml_dtypes.bfloat16)
        cst[:, 2 * NCOEF :] = eye
        in_maps.append({"x": np.ascontiguousarray(ph), "cst": cst})
    return in_maps


def gather_out(results):
    out = np.empty((SEQ_LEN, BSZ, EMBED_DIM), np.float32)
    for c in range(N_CORES):
        # [128, B, 4, J] phase-major -> [l = 4j+r, b, d]
        arr = results[c]["out"].astype(np.float32)
        out[:, :, c * D_PER : (c + 1) * D_PER] = arr.transpose(3, 2, 1, 0).reshape(
            SEQ_LEN, BSZ, D_PER
        )
    return out


def _run(inputs, **kwargs):
    nc = _get_nc()
    in_maps = make_in_maps(inputs)
    res = run_bass_kernel_spmd(nc, in_maps, core_ids=list(range(N_CORES)), **kwargs)
    return gather_out(res.results), res


def kernel(**inputs) -> np.ndarray:
    out, _ = _run(inputs)
    return out
